# revision 1
# baseline (speedup 1.0000x reference)
"""Trainium2 Bass kernel for a transformer block: MLA attention + top-2 MoE (8 experts).

Sharding (8 NeuronCores):
  Launch 1 (head-parallel attention): core c = (batch b=c//4, head-group
    g=c%4 of 4 heads). LN1 scale/bias are folded into host-scaled weights +
    per-partition projection biases. Causal attention uses a transposed-scores
    layout with softmax denominators accumulated via an augmented ones column.
    Partial out-projection attn_g @ Wo[g-rows] per core; host sums partials.
  Host: xnew = x + sum(partials); LN2; gate logits; top-2 softmax; per-expert
    token gather (the "all-to-all dispatch").
  Launch 2 (expert-parallel MLP): core e = expert e on its gathered tokens,
    combine weights folded in. Host scatter-adds ("combine").
"""

import numpy as np
import ml_dtypes

import concourse.bass as bass
import concourse.bacc as bacc
import concourse.mybir as mybir
from concourse.tile import TileContext
from concourse.masks import make_identity
from concourse.bass_utils import run_bass_kernel_spmd

F32 = mybir.dt.float32
BF16 = mybir.dt.bfloat16
AF = mybir.ActivationFunctionType

B, S, D = 2, 2048, 1024
H, DH, DL = 16, 64, 512
E, DFF, TOPK = 8, 2048, 2
HC = 4            # heads per core
HDC = HC * DH     # 256
EPS = 1e-5
NEG = -1.0e30

_cache = {}


def _ln_stats(nc, pool, xt, eps, p=128, d=D):
    """Returns (r, negmr): rstd and -mean*rstd, [p,1] fp32."""
    nsub = d // 512
    stats = pool.tile([p, nsub, 6], F32, name="ln_stats", tag="ln_stats")
    for i in range(nsub):
        nc.vector.bn_stats(out=stats[:, i, :], in_=xt[:, i * 512:(i + 1) * 512])
    mv = pool.tile([p, 2], F32, name="ln_mv", tag="ln_mv")
    nc.vector.bn_aggr(out=mv, in_=stats)
    r = pool.tile([p, 1], F32, name="ln_r", tag="ln_r")
    nc.scalar.activation(out=r, in_=mv[:, 1:2], func=AF.Sqrt, bias=eps, scale=1.0)
    nc.vector.reciprocal(out=r, in_=r)
    negmr = pool.tile([p, 1], F32, name="ln_negmr", tag="ln_negmr")
    nc.vector.tensor_scalar(out=negmr, in0=mv[:, 0:1], scalar1=r, scalar2=-1.0,
                            op0=mybir.AluOpType.mult, op1=mybir.AluOpType.mult)
    return r, negmr


def build_l1():
    nc = bacc.Bacc()
    xb = nc.dram_tensor("xb", [S, D], F32, kind="ExternalInput")
    wq = nc.dram_tensor("wq", [D, HDC], BF16, kind="ExternalInput")
    wdkv = nc.dram_tensor("wdkv", [D, DL], BF16, kind="ExternalInput")
    wukv = nc.dram_tensor("wukv", [DL, HDC], BF16, kind="ExternalInput")
    wo = nc.dram_tensor("wo", [HDC, D], BF16, kind="ExternalInput")
    maskc = nc.dram_tensor("maskc", [2 * 128, 256], F32, kind="ExternalInput")
    qbias = nc.dram_tensor("qbias", [128, 2], F32, kind="ExternalInput")
    cbias = nc.dram_tensor("cbias", [128, 4], F32, kind="ExternalInput")
    xpart = nc.dram_tensor("xpart", [S, D], F32, kind="ExternalOutput")

    with TileContext(nc) as tc:
        import contextlib
        with contextlib.ExitStack() as ctx:
            singles = ctx.enter_context(tc.tile_pool(name="singles", bufs=1))
            lnp = ctx.enter_context(tc.tile_pool(name="lnp", bufs=4))
            xio = ctx.enter_context(tc.tile_pool(name="xio", bufs=3))
            hbuf = ctx.enter_context(tc.tile_pool(name="hbuf", bufs=3))
            wpool = ctx.enter_context(tc.tile_pool(name="wpool", bufs=1))
            big = ctx.enter_context(tc.tile_pool(name="big", bufs=1))
            work = ctx.enter_context(tc.tile_pool(name="work", bufs=4))
            outp = ctx.enter_context(tc.tile_pool(name="outp", bufs=3))
            psA = ctx.enter_context(tc.tile_pool(name="psA", bufs=2, space="PSUM"))
            psS = ctx.enter_context(tc.tile_pool(name="psS", bufs=2, space="PSUM"))
            psO = ctx.enter_context(tc.tile_pool(name="psO", bufs=2, space="PSUM"))
            dram = ctx.enter_context(tc.tile_pool(name="dram", bufs=1, space="DRAM"))

            epst = singles.tile([128, 1], F32, name="epst", tag="epst")
            nc.vector.memset(epst, EPS)
            qbias_sb = singles.tile([128, 2], F32, name="qbias_sb", tag="qbias_sb")
            nc.sync.dma_start(out=qbias_sb, in_=qbias[:, :])
            cbias_sb = singles.tile([128, 4], F32, name="cbias_sb", tag="cbias_sb")
            nc.sync.dma_start(out=cbias_sb, in_=cbias[:, :])
            ident = singles.tile([128, 128], BF16, name="ident", tag="ident")
            make_identity(nc, ident)
            masks = singles.tile([128, 2, 256], F32, name="masks", tag="masks")
            nc.sync.dma_start(out=masks,
                              in_=maskc[:, :].rearrange("(v p) n -> p v n", v=2))

            hD = dram.tile([S, D], BF16, name="hD", tag="hD")
            kvD = dram.tile([S, HDC], BF16, name="kvD", tag="kvD")

            # --- Phase A: LN1 (scale/bias folded into weights); h -> hD ---
            for t in range(16):
                xt = xio.tile([128, D], F32, name="xt", tag="xt")
                nc.sync.dma_start(out=xt, in_=xb[t * 128:(t + 1) * 128, :])
                r, negmr = _ln_stats(nc, lnp, xt, epst)
                hbf = hbuf.tile([128, D], BF16, name="hbf", tag="hbf")
                nc.scalar.activation(out=hbf, in_=xt, func=AF.Identity,
                                     bias=negmr, scale=r)
                nc.sync.dma_start(out=hD[t * 128:(t + 1) * 128, :], in_=hbf)

            # --- Phase B: hT; ckvT; qT ---
            wq_sb = [wpool.tile([128, HDC], BF16, name=f"wq{d}", tag=f"wq{d}")
                     for d in range(8)]
            wdkv_sb = [wpool.tile([128, DL], BF16, name=f"wdkv{d}", tag=f"wdkv{d}")
                       for d in range(8)]
            for d in range(8):
                nc.sync.dma_start(out=wq_sb[d], in_=wq[d * 128:(d + 1) * 128, :])
                nc.sync.dma_start(out=wdkv_sb[d], in_=wdkv[d * 128:(d + 1) * 128, :])
            hT = [big.tile([128, S], BF16, name=f"hT{d}", tag=f"hT{d}")
                  for d in range(8)]
            for d in range(8):
                nc.sync.dma_start(out=hT[d], in_=hD[:, d * 128:(d + 1) * 128],
                                  transpose=True)
            ckvT = [big.tile([128, S], BF16, name=f"ckvT{i}", tag=f"ckvT{i}")
                    for i in range(4)]
            for rc in range(4):
                for dlt in range(4):
                    ps = psA.tile([128, 512], F32, name="psB", tag="psB")
                    for d in range(8):
                        nc.tensor.matmul(ps, wdkv_sb[d][:, dlt * 128:(dlt + 1) * 128],
                                         hT[d][:, rc * 512:(rc + 1) * 512],
                                         start=(d == 0), stop=(d == 7))
                    nc.scalar.activation(
                        out=ckvT[dlt][:, rc * 512:(rc + 1) * 512], in_=ps,
                        func=AF.Identity, bias=cbias_sb[:, dlt:dlt + 1], scale=1.0)
            qT = [big.tile([128, S], BF16, name=f"qT{i}", tag=f"qT{i}")
                  for i in range(2)]
            for ht in range(2):
                for rc in range(4):
                    ps = psA.tile([128, 512], F32, name="psB", tag="psB")
                    for d in range(8):
                        nc.tensor.matmul(ps, wq_sb[d][:, ht * 128:(ht + 1) * 128],
                                         hT[d][:, rc * 512:(rc + 1) * 512],
                                         start=(d == 0), stop=(d == 7))
                    nc.scalar.activation(
                        out=qT[ht][:, rc * 512:(rc + 1) * 512], in_=ps,
                        func=AF.Identity, bias=qbias_sb[:, ht:ht + 1], scale=1.0)

            # --- Phase C: kv natural (augmented ones col) -> kva, kvD ---
            wukv_sb = [wpool.tile([128, HDC], BF16, name=f"wukv{i}", tag=f"wukv{i}")
                       for i in range(4)]
            for i in range(4):
                nc.sync.dma_start(out=wukv_sb[i], in_=wukv[i * 128:(i + 1) * 128, :])
            kva = [big.tile([128, HC, DH + 1], BF16, name=f"kva{t}", tag=f"kva{t}")
                   for t in range(16)]
            for kt in range(16):
                nc.vector.memset(kva[kt][:, :, DH:DH + 1], 1.0)
                ps = psA.tile([128, 512], F32, name="psB", tag="psB")
                for dlt in range(4):
                    nc.tensor.matmul(ps[:, 0:HDC],
                                     ckvT[dlt][:, kt * 128:(kt + 1) * 128],
                                     wukv_sb[dlt],
                                     start=(dlt == 0), stop=(dlt == 3))
                nc.vector.tensor_copy(
                    out=kva[kt][:, :, 0:DH],
                    in_=ps[:, 0:HDC].rearrange("p (h d) -> p h d", h=HC))
                nc.sync.dma_start(out=kvD[kt * 128:(kt + 1) * 128, :],
                                  in_=kva[kt][:, :, 0:DH])

            # --- Phase D: kvT; causal attention ---
            kvT = [big.tile([128, S], BF16, name=f"kvT{i}", tag=f"kvT{i}")
                   for i in range(2)]
            for d in range(2):
                nc.sync.dma_start(out=kvT[d], in_=kvD[:, d * 128:(d + 1) * 128],
                                  transpose=True)
            attn_sb = [big.tile([128, HDC], BF16, name=f"attn{i}", tag=f"attn{i}")
                       for i in range(16)]
            for h in range(HC):
                tI, pO = h // 2, (h % 2) * 64
                for qp in range(8):
                    nkt = 2 * qp + 2
                    P0 = psO.tile([128, DH + 1], F32, name="Pacc", tag="Pacc")
                    P1 = psO.tile([128, DH + 1], F32, name="Pacc", tag="Pacc")
                    for g0 in range(0, nkt, 4):
                        gn = min(4, nkt - g0)
                        ps = psS.tile([128, 1024], F32, name="psS", tag="psS")
                        for kl in range(gn):
                            kt = g0 + kl
                            sl = ps[:, kl * 256:(kl + 1) * 256]
                            nc.tensor.matmul(
                                sl,
                                kvT[tI][pO:pO + 64, kt * 128:(kt + 1) * 128],
                                qT[tI][pO:pO + 64, qp * 256:(qp + 1) * 256],
                                start=True, stop=True)
                            if kt == 2 * qp:
                                nc.vector.tensor_add(out=sl, in0=sl,
                                                     in1=masks[:, 0, :])
                            elif kt == 2 * qp + 1:
                                nc.vector.tensor_add(out=sl, in0=sl,
                                                     in1=masks[:, 1, :])
                        pbT = work.tile([128, 1024], BF16, name="pbT", tag="pbT")
                        nc.scalar.activation(out=pbT[:, 0:gn * 256],
                                             in_=ps[:, 0:gn * 256], func=AF.Exp,
                                             scale=1.0 / (DH ** 0.5))
                        for kl in range(gn):
                            kt = g0 + kl
                            nc.tensor.matmul(P0, pbT[:, kl * 256:kl * 256 + 128],
                                             kva[kt][:, h, :],
                                             start=(kt == 0), stop=(kt == nkt - 1))
                            nc.tensor.matmul(P1, pbT[:, kl * 256 + 128:(kl + 1) * 256],
                                             kva[kt][:, h, :],
                                             start=(kt == 0), stop=(kt == nkt - 1))
                    for j, P in ((0, P0), (1, P1)):
                        rec = work.tile([128, 1], F32, name="rec", tag="rec")
                        nc.vector.reciprocal(out=rec, in_=P[:, DH:DH + 1])
                        nc.vector.tensor_scalar_mul(
                            out=attn_sb[2 * qp + j][:, h * DH:(h + 1) * DH],
                            in0=P[:, 0:DH], scalar1=rec)

            # --- Phase E: attnT via PE transpose; xpart = attn_g @ Wo[g] ---
            wo_sb = [wpool.tile([128, D], BF16, name=f"wo{i}", tag=f"wo{i}")
                     for i in range(2)]
            for i in range(2):
                nc.sync.dma_start(out=wo_sb[i], in_=wo[i * 128:(i + 1) * 128, :])
            attnT = [big.tile([128, S], BF16, name=f"attnT{i}", tag=f"attnT{i}")
                     for i in range(2)]
            for qt in range(16):
                for hd in range(2):
                    pt = psS.tile([128, 128], BF16, name="ptT", tag="psS")
                    nc.tensor.transpose(pt,
                                        attn_sb[qt][:, hd * 128:(hd + 1) * 128],
                                        ident)
                    nc.scalar.activation(out=attnT[hd][:, qt * 128:(qt + 1) * 128],
                                         in_=pt, func=AF.Copy)
            for qt in range(16):
                xp = outp.tile([128, D], F32, name="xp", tag="xp")
                for dh2 in range(2):
                    ps = psA.tile([128, 512], F32, name="psB", tag="psB")
                    for hd in range(2):
                        nc.tensor.matmul(ps, attnT[hd][:, qt * 128:(qt + 1) * 128],
                                         wo_sb[hd][:, dh2 * 512:(dh2 + 1) * 512],
                                         start=(hd == 0), stop=(hd == 1))
                    nc.scalar.activation(out=xp[:, dh2 * 512:(dh2 + 1) * 512],
                                         in_=ps, func=AF.Copy)
                nc.sync.dma_start(out=xpart[qt * 128:(qt + 1) * 128, :], in_=xp)
    nc.compile()
    return nc


def build_l2(capT: int):
    """Expert MLP on gathered tokens: yT = ((gelu(Xe@W1+b1) * w) @ W2 + w*b2)^T."""
    nc = bacc.Bacc()
    xeT = nc.dram_tensor("xeT", [D, capT], BF16, kind="ExternalInput")
    w1 = nc.dram_tensor("w1", [D, DFF], BF16, kind="ExternalInput")
    w2 = nc.dram_tensor("w2", [DFF, D], BF16, kind="ExternalInput")
    b1 = nc.dram_tensor("b1", [128, DFF // 128], F32, kind="ExternalInput")
    b2 = nc.dram_tensor("b2", [1, D], BF16, kind="ExternalInput")
    wrow = nc.dram_tensor("wrow", [1, capT], BF16, kind="ExternalInput")
    yT = nc.dram_tensor("yT", [D, capT], BF16, kind="ExternalOutput")

    chunks = []
    off = 0
    while off < capT:
        n = min(512, capT - off)
        chunks.append((off, n))
        off += n

    with TileContext(nc) as tc:
        import contextlib
        with contextlib.ExitStack() as ctx:
            singles = ctx.enter_context(tc.tile_pool(name="singles", bufs=1))
            wpool = ctx.enter_context(tc.tile_pool(name="wpool", bufs=1))
            big = ctx.enter_context(tc.tile_pool(name="big", bufs=1))
            outp = ctx.enter_context(tc.tile_pool(name="outp", bufs=3))
            psp = ctx.enter_context(tc.tile_pool(name="psp", bufs=4, space="PSUM"))

            b1s = singles.tile([128, DFF // 128], F32, name="b1s", tag="b1s")
            nc.sync.dma_start(out=b1s, in_=b1[:, :])
            b2s = singles.tile([1, D], BF16, name="b2s", tag="b2s")
            nc.sync.dma_start(out=b2s, in_=b2[:, :])
            ws = singles.tile([1, capT], BF16, name="ws", tag="ws")
            nc.sync.dma_start(out=ws, in_=wrow[:, :])
            wbc = singles.tile([128, capT], BF16, name="wbc", tag="wbc")
            nc.sync.dma_start(out=wbc, in_=wrow[:, :].to_broadcast([128, capT]))

            xe = [big.tile([128, capT], BF16, name=f"xe{d}", tag=f"xe{d}")
                  for d in range(8)]
            for d in range(8):
                nc.sync.dma_start(out=xe[d], in_=xeT[d * 128:(d + 1) * 128, :])
            w1s = [wpool.tile([128, DFF], BF16, name=f"w1s{d}", tag=f"w1s{d}")
                   for d in range(8)]
            for d in range(8):
                nc.sync.dma_start(out=w1s[d], in_=w1[d * 128:(d + 1) * 128, :])
            w2s = [wpool.tile([128, D], BF16, name=f"w2s{f}", tag=f"w2s{f}")
                   for f in range(16)]
            for f in range(16):
                nc.sync.dma_start(out=w2s[f], in_=w2[f * 128:(f + 1) * 128, :])

            hid = [big.tile([128, capT], BF16, name=f"hid{f}", tag=f"hid{f}")
                   for f in range(16)]
            for ft in range(16):
                for (off, n) in chunks:
                    ps = psp.tile([128, 512], F32, name="ps1", tag="ps1")
                    for d in range(8):
                        nc.tensor.matmul(ps[:, 0:n],
                                         w1s[d][:, ft * 128:(ft + 1) * 128],
                                         xe[d][:, off:off + n],
                                         start=(d == 0), stop=(d == 7))
                    nc.scalar.activation(out=hid[ft][:, off:off + n],
                                         in_=ps[:, 0:n], func=AF.Gelu,
                                         bias=b1s[:, ft:ft + 1])
                nc.vector.tensor_mul(out=hid[ft], in0=hid[ft], in1=wbc)

            for dt in range(8):
                for (off, n) in chunks:
                    ps = psp.tile([128, 512], F32, name="ps2", tag="ps2")
                    for ft in range(16):
                        nc.tensor.matmul(ps[:, 0:n],
                                         w2s[ft][:, dt * 128:(dt + 1) * 128],
                                         hid[ft][:, off:off + n],
                                         start=(ft == 0), stop=False)
                    nc.tensor.matmul(ps[:, 0:n],
                                     b2s[:, dt * 128:(dt + 1) * 128],
                                     ws[:, off:off + n],
                                     start=False, stop=True)
                    ot = outp.tile([128, 512], BF16, name="ot", tag="ot")
                    nc.scalar.activation(out=ot[:, 0:n], in_=ps[:, 0:n], func=AF.Copy)
                    nc.sync.dma_start(out=yT[dt * 128:(dt + 1) * 128, off:off + n],
                                      in_=ot[:, 0:n])
    nc.compile()
    return nc


def _bf(a):
    return np.ascontiguousarray(np.asarray(a).astype(ml_dtypes.bfloat16))


def _f32c(a):
    return np.ascontiguousarray(np.asarray(a, np.float32))


def kernel(x, mask, ln1_scale, ln1_bias, Wq, Wdkv, Wukv, Wo,
           ln2_scale, ln2_bias, Wgate, bgate, We1, be1, We2, be2,
           _collect=None):
    x = np.asarray(x, np.float32)
    g1v = np.asarray(ln1_scale, np.float32)
    b1v = np.asarray(ln1_bias, np.float32)
    Wq_f = np.asarray(Wq, np.float32)
    Wdkv_f = np.asarray(Wdkv, np.float32)

    # fold LN1 scale into projection weights; LN1 bias becomes channel biases
    Wq_s = g1v[:, None] * Wq_f
    Wdkv_s = g1v[:, None] * Wdkv_f
    qb_full = b1v @ Wq_f          # (H*DH,)
    cb = b1v @ Wdkv_f             # (DL,)

    ii = np.arange(128)[:, None]
    jj = np.arange(256)[None, :]
    tri = np.where(jj[:, :128] >= ii, 0.0, NEG).astype(np.float32)
    m0 = np.concatenate([tri, np.zeros((128, 128), np.float32)], axis=1)
    m1 = np.concatenate([np.full((128, 128), NEG, np.float32), tri], axis=1)
    maskc = np.ascontiguousarray(np.concatenate([m0, m1], axis=0))

    wdkv_b = _bf(Wdkv_s)
    cbias = np.ascontiguousarray(cb.reshape(4, 128).T.astype(np.float32))
    l1_maps = []
    for c in range(8):
        b, g = c // 4, c % 4
        cs = slice(g * HDC, (g + 1) * HDC)
        l1_maps.append({
            "xb": _f32c(x[b]),
            "wq": _bf(Wq_s[:, cs]),
            "wdkv": wdkv_b,
            "wukv": _bf(np.asarray(Wukv)[:, cs]),
            "wo": _bf(np.asarray(Wo)[cs, :]),
            "maskc": maskc,
            "qbias": np.ascontiguousarray(
                qb_full[cs].reshape(2, 128).T.astype(np.float32)),
            "cbias": cbias,
        })

    if "l1" not in _cache:
        _cache["l1"] = build_l1()
    r1 = run_bass_kernel_spmd(_cache["l1"], l1_maps, core_ids=list(range(8)))
    if _collect is not None:
        _collect["r1"] = r1

    xnew = x.copy().reshape(B, S, D)
    for c in range(8):
        xnew[c // 4] += r1.results[c]["xpart"]
    xf = xnew.reshape(B * S, D)

    # LN2 + gate on host (fp32)
    mu = xf.mean(axis=1, keepdims=True)
    var = ((xf - mu) ** 2).mean(axis=1, keepdims=True)
    h2 = ((xf - mu) / np.sqrt(var + EPS) * np.asarray(ln2_scale, np.float32)
          + np.asarray(ln2_bias, np.float32)).astype(np.float32)
    logits = h2 @ np.asarray(Wgate, np.float32) + np.asarray(bgate, np.float32)
    order = np.argsort(-logits, axis=1, kind="stable")[:, :TOPK]
    tv = np.take_along_axis(logits, order, axis=1)
    ex = np.exp(tv - tv.max(axis=1, keepdims=True))
    wtop = (ex / ex.sum(axis=1, keepdims=True)).astype(np.float32)

    idxs, wts = [], []
    for e in range(E):
        m_e = (order == e)
        rows = np.nonzero(m_e.any(axis=1))[0]
        w_e = (wtop * m_e).sum(axis=1)[rows]
        idxs.append(rows)
        wts.append(w_e.astype(np.float32))
    maxc = max(len(r) for r in idxs)
    capT = max(512, ((maxc + 127) // 128) * 128)

    h2b = h2.astype(ml_dtypes.bfloat16)
    w1_b, w2_b = np.asarray(We1), np.asarray(We2)
    be1_f, be2_b = np.asarray(be1, np.float32), np.asarray(be2)
    l2_maps = []
    for e in range(E):
        n = len(idxs[e])
        xeT = np.zeros((D, capT), ml_dtypes.bfloat16)
        xeT[:, :n] = h2b[idxs[e]].T
        wr = np.zeros((1, capT), ml_dtypes.bfloat16)
        wr[0, :n] = wts[e].astype(ml_dtypes.bfloat16)
        l2_maps.append({
            "xeT": np.ascontiguousarray(xeT),
            "w1": _bf(w1_b[e]),
            "w2": _bf(w2_b[e]),
            "b1": np.ascontiguousarray(be1_f[e].reshape(DFF // 128, 128).T),
            "b2": _bf(be2_b[e].reshape(1, D)),
            "wrow": wr,
        })

    key = ("l2", capT)
    if key not in _cache:
        _cache[key] = build_l2(capT)
    r2 = run_bass_kernel_spmd(_cache[key], l2_maps, core_ids=list(range(8)))
    if _collect is not None:
        _collect["r2"] = r2

    out = xf.copy()
    for e in range(E):
        n = len(idxs[e])
        out[idxs[e]] += r2.results[e]["yT"][:, :n].T.astype(np.float32)
    return out.reshape(B, S, D).astype(np.float32)



# revision 20
# speedup vs baseline: 1.2842x; 1.2842x over previous
"""Trainium2 Bass kernel for a transformer block: MLA attention + top-2 MoE (8 experts).

Sharding (8 NeuronCores):
  Launch 1 (head-parallel attention): core c = (batch b=c//4, head-group
    g=c%4 of 4 heads). LN1 scale/bias are folded into host-scaled weights +
    per-partition projection biases. Causal attention uses a transposed-scores
    layout with softmax denominators accumulated via an augmented ones column.
    Partial out-projection attn_g @ Wo[g-rows] per core; host sums partials.
  Host: xnew = x + sum(partials); LN2; gate logits; top-2 softmax; per-expert
    token gather (the "all-to-all dispatch").
  Launch 2 (expert-parallel MLP): core e = expert e on its gathered tokens,
    combine weights folded in. Host scatter-adds ("combine").
"""

import numpy as np
import ml_dtypes

import concourse.bass as bass
import concourse.bacc as bacc
import concourse.mybir as mybir
from concourse.tile import TileContext
from concourse.masks import make_identity
from concourse.bass_utils import run_bass_kernel_spmd

F32 = mybir.dt.float32
BF16 = mybir.dt.bfloat16
AF = mybir.ActivationFunctionType

B, S, D = 2, 2048, 1024
H, DH, DL = 16, 64, 512
E, DFF, TOPK = 8, 2048, 2
HC = 4            # heads per core
HDC = HC * DH     # 256
EPS = 1e-5
NEG = -1.0e30

_cache = {}


# l1 fp8 scales: h ×SXA, Wq/Wdkv ×SWP (q/ckv psum = SXA*SWP * true)
SXA, SWP = 4.0, 64.0


def build_l1():
    """Head-parallel MLA attention. Host pre-computes LN1 and passes hT fp8.

    Per core: q/ckv projections in fp8 DoubleRow; kv up-projection in bf16
    (both layouts computed on PE, no DRAM roundtrip); causal softmax with
    multiplicative 0/1 masks applied to exp(scores); denominators via an
    augmented ones column; out-projection partials DMAed straight from PSUM.
    """
    nc = bacc.Bacc()
    hT8 = nc.dram_tensor("hT8", [D, S], F8, kind="ExternalInput")
    wq = nc.dram_tensor("wq", [D, HDC], F8, kind="ExternalInput")
    wdkv = nc.dram_tensor("wdkv", [D, DL], F8, kind="ExternalInput")
    wukv = nc.dram_tensor("wukv", [DL, HDC], BF16, kind="ExternalInput")
    wo = nc.dram_tensor("wo", [HDC, D], BF16, kind="ExternalInput")
    mmask = nc.dram_tensor("mmask", [2 * 128, 256], BF16, kind="ExternalInput")
    xpart = nc.dram_tensor("xpart", [S, D], F32, kind="ExternalOutput")

    PRJ = 1.0 / (SXA * SWP)

    with TileContext(nc) as tc:
        import contextlib
        with contextlib.ExitStack() as ctx:
            singles = ctx.enter_context(tc.tile_pool(name="singles", bufs=1))
            wpool = ctx.enter_context(tc.tile_pool(name="wpool", bufs=1))
            big = ctx.enter_context(tc.tile_pool(name="big", bufs=1))
            work = ctx.enter_context(tc.tile_pool(name="work", bufs=4))
            psA = ctx.enter_context(tc.tile_pool(name="psA", bufs=2, space="PSUM"))
            psS = ctx.enter_context(tc.tile_pool(name="psS", bufs=2, space="PSUM"))
            psO = ctx.enter_context(tc.tile_pool(name="psO", bufs=2, space="PSUM"))

            # loads, critical-path first
            hTs = [big.tile([128, 8, 512], F8, name=f"hT{rc}", tag=f"hT{rc}")
                   for rc in range(4)]

            def _load_h(rc):
                nc.sync.dma_start(
                    out=hTs[rc],
                    in_=hT8[:, rc * 512:(rc + 1) * 512].rearrange(
                        "(j s p) n -> p (j s) n", j=4, s=2))

            _load_h(0)
            wq_sb = wpool.tile([128, 8, HDC], F8, name="wq", tag="wq")
            nc.sync.dma_start(
                out=wq_sb, in_=wq[:, :].rearrange("(j s p) n -> p (j s) n",
                                                  j=4, s=2))
            wdkv_sb = wpool.tile([128, 8, DL], F8, name="wdkv", tag="wdkv")
            nc.sync.dma_start(
                out=wdkv_sb, in_=wdkv[:, :].rearrange("(j s p) n -> p (j s) n",
                                                      j=4, s=2))
            wukv_sb = [wpool.tile([128, HDC], BF16, name=f"wukv{i}",
                                  tag=f"wukv{i}") for i in range(4)]
            for i in range(4):
                nc.sync.dma_start(out=wukv_sb[i],
                                  in_=wukv[i * 128:(i + 1) * 128, :])
            masks = singles.tile([128, 2, 256], BF16, name="masks", tag="masks")
            nc.sync.dma_start(out=masks,
                              in_=mmask[:, :].rearrange("(v p) n -> p v n", v=2))
            ident = singles.tile([128, 128], BF16, name="ident", tag="ident")
            make_identity(nc, ident)
            for rc in range(1, 4):
                _load_h(rc)
            wo_sb = [wpool.tile([128, D], BF16, name=f"wo{i}", tag=f"wo{i}")
                     for i in range(2)]
            for i in range(2):
                nc.sync.dma_start(out=wo_sb[i], in_=wo[i * 128:(i + 1) * 128, :])

            qT = [big.tile([128, S], BF16, name=f"qT{i}", tag=f"qT{i}")
                  for i in range(2)]
            ckvT = [big.tile([128, S], BF16, name=f"ckvT{i}", tag=f"ckvT{i}")
                    for i in range(4)]
            kvT = [big.tile([128, S], BF16, name=f"kvT{i}", tag=f"kvT{i}")
                   for i in range(2)]
            kva = [big.tile([128, HC, DH + 1], BF16, name=f"kva{t}",
                            tag=f"kva{t}") for t in range(16)]
            attn_sb = [big.tile([128, HDC], BF16, name=f"attn{i}", tag=f"attn{i}")
                       for i in range(16)]
            attnT = [big.tile([128, S], BF16, name=f"attnT{i}", tag=f"attnT{i}")
                     for i in range(2)]

            def proj(rc):
                cs = slice(rc * 512, (rc + 1) * 512)
                # ckvT = (h @ Wdkv)^T in bf16, scaled back to true values
                for dlt in range(4):
                    ps = psA.tile([128, 512], F32, name="psB", tag="psB")
                    for j in range(4):
                        nc.tensor.matmul(ps,
                                         wdkv_sb[:, 2 * j:2 * j + 2,
                                                 dlt * 128:(dlt + 1) * 128],
                                         hTs[rc][:, 2 * j:2 * j + 2, :],
                                         start=(j == 0), stop=(j == 3),
                                         perf_mode=DR)
                    nc.vector.tensor_scalar(out=ckvT[dlt][:, cs], in0=ps,
                                            scalar1=PRJ, scalar2=None,
                                            op0=mybir.AluOpType.mult)
                for ht in range(2):
                    ps = psA.tile([128, 512], F32, name="psB", tag="psB")
                    for j in range(4):
                        nc.tensor.matmul(ps,
                                         wq_sb[:, 2 * j:2 * j + 2,
                                               ht * 128:(ht + 1) * 128],
                                         hTs[rc][:, 2 * j:2 * j + 2, :],
                                         start=(j == 0), stop=(j == 3),
                                         perf_mode=DR)
                    nc.vector.tensor_scalar(out=qT[ht][:, cs], in0=ps,
                                            scalar1=PRJ, scalar2=None,
                                            op0=mybir.AluOpType.mult)
                # kv in both layouts (all bf16, true values)
                for ht in range(2):
                    ps = psA.tile([128, 512], F32, name="psB", tag="psB")
                    for dlt in range(4):
                        nc.tensor.matmul(ps,
                                         wukv_sb[dlt][:, ht * 128:(ht + 1) * 128],
                                         ckvT[dlt][:, cs],
                                         start=(dlt == 0), stop=(dlt == 3))
                    nc.vector.tensor_copy(out=kvT[ht][:, cs], in_=ps)
                for kt in range(4 * rc, 4 * rc + 4):
                    nc.gpsimd.memset(kva[kt][:, :, DH:DH + 1], 1.0)
                    ps = psA.tile([128, 512], F32, name="psB", tag="psB")
                    for dlt in range(4):
                        nc.tensor.matmul(ps[:, 0:HDC],
                                         ckvT[dlt][:, kt * 128:(kt + 1) * 128],
                                         wukv_sb[dlt],
                                         start=(dlt == 0), stop=(dlt == 3))
                    nc.vector.tensor_copy(
                        out=kva[kt][:, :, 0:DH],
                        in_=ps[:, 0:HDC].rearrange("p (h d) -> p h d", h=HC))

            def attn(qp):
                nkt = 2 * qp + 2
                for h in range(HC):
                    tI, pO = h // 2, (h % 2) * 64
                    P0 = psO.tile([128, DH + 1], F32, name="Pacc", tag="Pacc")
                    P1 = psO.tile([128, DH + 1], F32, name="Pacc", tag="Pacc")
                    for g0 in range(0, nkt, 4):
                        gn = min(4, nkt - g0)
                        ps = psS.tile([128, 1024], F32, name="psS", tag="psS")
                        for kl in range(gn):
                            kt = g0 + kl
                            nc.tensor.matmul(
                                ps[:, kl * 256:(kl + 1) * 256],
                                kvT[tI][pO:pO + 64, kt * 128:(kt + 1) * 128],
                                qT[tI][pO:pO + 64, qp * 256:(qp + 1) * 256],
                                start=True, stop=True)
                        pbT = work.tile([128, 1024], BF16, name="pbT", tag="pbT")
                        nc.scalar.activation(out=pbT[:, 0:gn * 256],
                                             in_=ps[:, 0:gn * 256], func=AF.Exp,
                                             scale=1.0 / (DH ** 0.5))
                        for kl in range(gn):
                            kt = g0 + kl
                            if kt >= 2 * qp:  # diagonal: zero masked probs
                                eng = nc.vector if h % 2 == 0 else nc.gpsimd
                                eng.tensor_mul(
                                    out=pbT[:, kl * 256:(kl + 1) * 256],
                                    in0=pbT[:, kl * 256:(kl + 1) * 256],
                                    in1=masks[:, kt - 2 * qp, :])
                            nc.tensor.matmul(P0, pbT[:, kl * 256:kl * 256 + 128],
                                             kva[kt][:, h, :],
                                             start=(kt == 0), stop=(kt == nkt - 1))
                            nc.tensor.matmul(P1,
                                             pbT[:, kl * 256 + 128:(kl + 1) * 256],
                                             kva[kt][:, h, :],
                                             start=(kt == 0), stop=(kt == nkt - 1))
                    for j, P in ((0, P0), (1, P1)):
                        rec = work.tile([128, 1], F32, name="rec", tag="rec")
                        nc.vector.reciprocal(out=rec, in_=P[:, DH:DH + 1])
                        nc.vector.tensor_scalar_mul(
                            out=attn_sb[2 * qp + j][:, h * DH:(h + 1) * DH],
                            in0=P[:, 0:DH], scalar1=rec)

            def post(qp):
                for qt in (2 * qp, 2 * qp + 1):
                    for hd in range(2):
                        pt = psO.tile([128, 128], BF16, name="ptT", tag="Pacc")
                        nc.tensor.transpose(
                            pt, attn_sb[qt][:, hd * 128:(hd + 1) * 128], ident)
                        nc.vector.tensor_copy(
                            out=attnT[hd][:, qt * 128:(qt + 1) * 128], in_=pt)
                    for dh2 in range(2):
                        ps = psA.tile([128, 512], F32, name="psB", tag="psB")
                        for hd in range(2):
                            nc.tensor.matmul(
                                ps, attnT[hd][:, qt * 128:(qt + 1) * 128],
                                wo_sb[hd][:, dh2 * 512:(dh2 + 1) * 512],
                                start=(hd == 0), stop=(hd == 1))
                        nc.sync.dma_start(
                            out=xpart[qt * 128:(qt + 1) * 128,
                                      dh2 * 512:(dh2 + 1) * 512],
                            in_=ps)

            for rc in range(4):
                proj(rc)
                for qp in (2 * rc, 2 * rc + 1):
                    attn(qp)
                    post(qp)
    nc.compile()
    return nc


F8 = mybir.dt.float8e4
DR = mybir.MatmulPerfMode.DoubleRow

# fp8 scale factors (powers of 2, folded back out via activation scale)
SX, SW1, SH, SW2 = 4.0, 64.0, 16.0, 32.0


def build_l2(capT: int):
    """Expert MLP on gathered tokens, fp8e4 DoubleRow matmuls.

    yT = gelu(Xe@W1 + b1) @ W2 / SW2  (un-combined); host applies the top-k
    combine weight and adds w*b2 during scatter. Xe is pre-scaled by SX,
    W1 by SW1, W2 by SW2 on the host.
    """
    nc = bacc.Bacc()
    xeT = nc.dram_tensor("xeT", [D, capT], F8, kind="ExternalInput")
    w1 = nc.dram_tensor("w1", [D, DFF], F8, kind="ExternalInput")
    w2 = nc.dram_tensor("w2", [DFF, D], F8, kind="ExternalInput")
    b1 = nc.dram_tensor("b1", [128, DFF // 128], F32, kind="ExternalInput")
    yT = nc.dram_tensor("yT", [D, capT], BF16, kind="ExternalOutput")

    chunks = [(0, min(256, capT))]
    off = chunks[0][1]
    while off < capT:
        n = min(512, capT - off)
        chunks.append((off, n))
        off += n

    with TileContext(nc) as tc:
        import contextlib
        with contextlib.ExitStack() as ctx:
            singles = ctx.enter_context(tc.tile_pool(name="singles", bufs=1))
            wpool = ctx.enter_context(tc.tile_pool(name="wpool", bufs=1))
            big = ctx.enter_context(tc.tile_pool(name="big", bufs=1))
            outp = ctx.enter_context(tc.tile_pool(name="outp", bufs=4))
            psp = ctx.enter_context(tc.tile_pool(name="psp", bufs=4, space="PSUM"))
            psq = ctx.enter_context(tc.tile_pool(name="psq", bufs=4, space="PSUM"))

            # single-DMA loads: xe per chunk, w1 in two ft-halves, w2 whole,
            # issued in critical-path-first order
            xec = [big.tile([128, 8, n], F8, name=f"xe{c}", tag=f"xe{c}")
                   for c, (off, n) in enumerate(chunks)]
            w1s = [wpool.tile([128, 8, DFF // 4], F8, name=f"w1s{h}",
                              tag=f"w1s{h}") for h in range(4)]
            w2s = wpool.tile([128, 16, D], F8, name="w2s", tag="w2s")
            b1s = singles.tile([128, DFF // 128], F32, name="b1s", tag="b1s")

            def _load_xe(c):
                off, n = chunks[c]
                nc.sync.dma_start(
                    out=xec[c],
                    in_=xeT[:, off:off + n].rearrange(
                        "(j s p) n -> p (j s) n", j=4, s=2))

            _load_xe(0)
            q = DFF // 4
            nc.sync.dma_start(
                out=w1s[0],
                in_=w1[:, 0:q].rearrange("(j s p) n -> p (j s) n", j=4, s=2))
            nc.sync.dma_start(out=b1s, in_=b1[:, :])
            for h in range(1, 4):
                nc.sync.dma_start(
                    out=w1s[h],
                    in_=w1[:, h * q:(h + 1) * q].rearrange(
                        "(j s p) n -> p (j s) n", j=4, s=2))
            for c in range(1, len(chunks)):
                _load_xe(c)
            nc.sync.dma_start(
                out=w2s,
                in_=w2[:, :].rearrange("(j s p) n -> p (j s) n", j=8, s=2))

            hid2 = [[big.tile([128, 2, n], F8, name=f"hid2_{c}_{j}",
                              tag=f"hid2_{c}_{j}") for j in range(8)]
                    for c, (off, n) in enumerate(chunks)]
            for c, (off, n) in enumerate(chunks):
                for ft in range(16):
                    h, fl = ft // 4, ft % 4
                    ps = psp.tile([128, 512], F32, name="ps1", tag="ps1")
                    for j in range(4):
                        nc.tensor.matmul(ps[:, 0:n],
                                         w1s[h][:, 2 * j:2 * j + 2,
                                                fl * 128:(fl + 1) * 128],
                                         xec[c][:, 2 * j:2 * j + 2, 0:n],
                                         start=(j == 0), stop=(j == 3),
                                         perf_mode=DR)
                    nc.scalar.activation(out=hid2[c][ft // 2][:, ft % 2, 0:n],
                                         in_=ps[:, 0:n], func=AF.Gelu,
                                         bias=b1s[:, ft:ft + 1],
                                         scale=1.0 / (SX * SW1))
            for c, (off, n) in enumerate(chunks):
                for dt in range(8):
                    ps = psq.tile([128, 512], F32, name="ps2", tag="ps2")
                    for j in range(8):
                        nc.tensor.matmul(ps[:, 0:n],
                                         w2s[:, 2 * j:2 * j + 2,
                                             dt * 128:(dt + 1) * 128],
                                         hid2[c][j][:, :, 0:n],
                                         start=(j == 0), stop=(j == 7),
                                         perf_mode=DR)
                    ot = outp.tile([128, 512], BF16, name="ot", tag="ot")
                    if dt % 2 == 0:
                        nc.vector.tensor_scalar(out=ot[:, 0:n], in0=ps[:, 0:n],
                                                scalar1=1.0 / SW2, scalar2=None,
                                                op0=mybir.AluOpType.mult)
                    else:
                        nc.scalar.activation(out=ot[:, 0:n], in_=ps[:, 0:n],
                                             func=AF.Copy, scale=1.0 / SW2)
                    nc.sync.dma_start(out=yT[dt * 128:(dt + 1) * 128, off:off + n],
                                      in_=ot[:, 0:n])
    nc.compile()
    return nc


def _bf(a):
    return np.ascontiguousarray(np.asarray(a).astype(ml_dtypes.bfloat16))


def _f8(a, scale):
    a = np.asarray(a, np.float32) * scale
    np.clip(a, -240.0, 240.0, out=a)
    return np.ascontiguousarray(a.astype(ml_dtypes.float8_e4m3))


def _f32c(a):
    return np.ascontiguousarray(np.asarray(a, np.float32))


def kernel(x, mask, ln1_scale, ln1_bias, Wq, Wdkv, Wukv, Wo,
           ln2_scale, ln2_bias, Wgate, bgate, We1, be1, We2, be2,
           _collect=None):
    x = np.asarray(x, np.float32)
    g1v = np.asarray(ln1_scale, np.float32)
    b1v = np.asarray(ln1_bias, np.float32)
    Wq_f = np.asarray(Wq, np.float32)
    Wdkv_f = np.asarray(Wdkv, np.float32)

    # fold LN1 scale into projection weights; LN1 bias becomes channel biases
    Wq_s = g1v[:, None] * Wq_f
    Wdkv_s = g1v[:, None] * Wdkv_f
    qb_full = b1v @ Wq_f          # (H*DH,)
    cb = b1v @ Wdkv_f             # (DL,)

    ii = np.arange(128)[:, None]
    jj = np.arange(256)[None, :]
    tri = np.where(jj[:, :128] >= ii, 0.0, NEG).astype(np.float32)
    m0 = np.concatenate([tri, np.zeros((128, 128), np.float32)], axis=1)
    m1 = np.concatenate([np.full((128, 128), NEG, np.float32), tri], axis=1)
    maskc = np.ascontiguousarray(np.concatenate([m0, m1], axis=0))

    wdkv_b = _bf(Wdkv_s)
    cbias = np.ascontiguousarray(cb.reshape(4, 128).T.astype(np.float32))
    l1_maps = []
    for c in range(8):
        b, g = c // 4, c % 4
        cs = slice(g * HDC, (g + 1) * HDC)
        l1_maps.append({
            "xb": _f32c(x[b]),
            "wq": _bf(Wq_s[:, cs]),
            "wdkv": wdkv_b,
            "wukv": _bf(np.asarray(Wukv)[:, cs]),
            "wo": _bf(np.asarray(Wo)[cs, :]),
            "maskc": maskc,
            "qbias": np.ascontiguousarray(
                qb_full[cs].reshape(2, 128).T.astype(np.float32)),
            "cbias": cbias,
        })

    if "l1" not in _cache:
        _cache["l1"] = build_l1()
    r1 = run_bass_kernel_spmd(_cache["l1"], l1_maps, core_ids=list(range(8)))
    if _collect is not None:
        _collect["r1"] = r1

    xnew = x.copy().reshape(B, S, D)
    for c in range(8):
        xnew[c // 4] += r1.results[c]["xpart"]
    xf = xnew.reshape(B * S, D)

    # LN2 + gate on host (fp32)
    mu = xf.mean(axis=1, keepdims=True)
    var = ((xf - mu) ** 2).mean(axis=1, keepdims=True)
    h2 = ((xf - mu) / np.sqrt(var + EPS) * np.asarray(ln2_scale, np.float32)
          + np.asarray(ln2_bias, np.float32)).astype(np.float32)
    logits = h2 @ np.asarray(Wgate, np.float32) + np.asarray(bgate, np.float32)
    order = np.argsort(-logits, axis=1, kind="stable")[:, :TOPK]
    tv = np.take_along_axis(logits, order, axis=1)
    ex = np.exp(tv - tv.max(axis=1, keepdims=True))
    wtop = (ex / ex.sum(axis=1, keepdims=True)).astype(np.float32)

    idxs, wts = [], []
    for e in range(E):
        m_e = (order == e)
        rows = np.nonzero(m_e.any(axis=1))[0]
        w_e = (wtop * m_e).sum(axis=1)[rows]
        idxs.append(rows)
        wts.append(w_e.astype(np.float32))
    maxc = max(len(r) for r in idxs)
    capT = max(512, ((maxc + 127) // 128) * 128)

    w1_b, w2_b = np.asarray(We1), np.asarray(We2)
    be1_f, be2_f = np.asarray(be1, np.float32), np.asarray(be2, np.float32)
    l2_maps = []
    for e in range(E):
        n = len(idxs[e])
        xeT = np.zeros((D, capT), ml_dtypes.float8_e4m3)
        xeT[:, :n] = _f8(h2[idxs[e]].T, SX)
        l2_maps.append({
            "xeT": np.ascontiguousarray(xeT),
            "w1": _f8(w1_b[e], SW1),
            "w2": _f8(w2_b[e], SW2),
            "b1": np.ascontiguousarray(be1_f[e].reshape(DFF // 128, 128).T),
        })

    key = ("l2", capT)
    if key not in _cache:
        _cache[key] = build_l2(capT)
    r2 = run_bass_kernel_spmd(_cache[key], l2_maps, core_ids=list(range(8)))
    if _collect is not None:
        _collect["r2"] = r2

    out = xf.copy()
    for e in range(E):
        n = len(idxs[e])
        out[idxs[e]] += wts[e][:, None] * (
            r2.results[e]["yT"][:, :n].T.astype(np.float32)
            + be2_f[e][None, :])
    return out.reshape(B, S, D).astype(np.float32)



# revision 49
# speedup vs baseline: 2.3786x; 1.8522x over previous
"""Trainium2 Bass kernel for a transformer block: MLA attention + top-2 MoE (8 experts).

Sharding (8 NeuronCores):
  Launch 1 (head-parallel attention): core c = (batch b=c//4, head-group
    g=c%4 of 4 heads). LN1 scale/bias are folded into host-scaled weights +
    per-partition projection biases. Causal attention uses a transposed-scores
    layout with softmax denominators accumulated via an augmented ones column.
    Partial out-projection attn_g @ Wo[g-rows] per core; host sums partials.
  Host: xnew = x + sum(partials); LN2; gate logits; top-2 softmax; per-expert
    token gather (the "all-to-all dispatch").
  Launch 2 (expert-parallel MLP): core e = expert e on its gathered tokens,
    combine weights folded in. Host scatter-adds ("combine").
"""

import numpy as np
import ml_dtypes

import concourse.bass as bass
import concourse.bacc as bacc
import concourse.mybir as mybir
from concourse.tile import TileContext
from concourse.masks import make_identity
from concourse.bass_utils import run_bass_kernel_spmd

F32 = mybir.dt.float32
BF16 = mybir.dt.bfloat16
AF = mybir.ActivationFunctionType

B, S, D = 2, 2048, 1024
H, DH, DL = 16, 64, 512
E, DFF, TOPK = 8, 2048, 2
HC = 4            # heads per core
HDC = HC * DH     # 256
EPS = 1e-5
NEG = -1.0e30

_cache = {}


# l1 fp8 scales: h ×SXA, Wq/Wdkv ×SWP (q/ckv psum = SXA*SWP * true)
SXA, SWP = 4.0, 64.0
GSZ = 4     # k-tiles per softmax group (psS bank budget)
DPIPE = 2   # scores-ahead-of-PV software pipeline depth


def build_l1():
    """Head-parallel MLA attention. Host pre-computes LN1 and passes hT fp8.

    Per core: q/ckv projections in fp8 DoubleRow; kv up-projection in bf16
    (both layouts computed on PE, no DRAM roundtrip); causal softmax with
    multiplicative 0/1 masks applied to exp(scores); denominators via an
    augmented ones column; out-projection partials DMAed straight from PSUM.
    """
    nc = bacc.Bacc()
    hT8 = nc.dram_tensor("hT8", [D, S], F8, kind="ExternalInput")
    wq = nc.dram_tensor("wq", [D, HDC], F8, kind="ExternalInput")
    wdkv = nc.dram_tensor("wdkv", [D, DL], F8, kind="ExternalInput")
    wukv = nc.dram_tensor("wukv", [DL, HDC], BF16, kind="ExternalInput")
    wo = nc.dram_tensor("wo", [HDC, D], BF16, kind="ExternalInput")
    mmask = nc.dram_tensor("mmask", [2 * 128, 256], BF16, kind="ExternalInput")
    xpart = nc.dram_tensor("xpart", [S, D], BF16, kind="ExternalOutput")

    PRJ = 1.0 / (SXA * SWP)

    with TileContext(nc) as tc:
        import contextlib
        with contextlib.ExitStack() as ctx:
            singles = ctx.enter_context(tc.tile_pool(name="singles", bufs=1))
            wpool = ctx.enter_context(tc.tile_pool(name="wpool", bufs=1))
            big = ctx.enter_context(tc.tile_pool(name="big", bufs=1))
            work = ctx.enter_context(tc.tile_pool(name="work", bufs=6))
            psA = ctx.enter_context(tc.tile_pool(name="psA", bufs=2, space="PSUM"))
            psS = ctx.enter_context(tc.tile_pool(name="psS", bufs=2, space="PSUM"))
            psO = ctx.enter_context(tc.tile_pool(name="psO", bufs=2, space="PSUM"))

            # loads, critical-path first
            hTs = [big.tile([128, 8, 512], F8, name=f"hT{rc}", tag=f"hT{rc}")
                   for rc in range(4)]

            def _load_h(rc):
                nc.sync.dma_start(
                    out=hTs[rc],
                    in_=hT8[:, rc * 512:(rc + 1) * 512].rearrange(
                        "(j s p) n -> p (j s) n", j=4, s=2))

            _load_h(0)
            wdkv_sb = wpool.tile([128, 8, DL], F8, name="wdkv", tag="wdkv")
            nc.sync.dma_start(
                out=wdkv_sb, in_=wdkv[:, :].rearrange("(j s p) n -> p (j s) n",
                                                      j=4, s=2))
            wukv_sb = [wpool.tile([128, HDC], BF16, name=f"wukv{i}",
                                  tag=f"wukv{i}") for i in range(4)]
            for i in range(4):
                nc.sync.dma_start(out=wukv_sb[i],
                                  in_=wukv[i * 128:(i + 1) * 128, :])
            wq_sb = wpool.tile([128, 8, HDC], F8, name="wq", tag="wq")
            nc.sync.dma_start(
                out=wq_sb, in_=wq[:, :].rearrange("(j s p) n -> p (j s) n",
                                                  j=4, s=2))
            masks = singles.tile([128, 2, 256], BF16, name="masks", tag="masks")
            nc.sync.dma_start(out=masks,
                              in_=mmask[:, :].rearrange("(v p) n -> p v n", v=2))
            ident = singles.tile([128, 128], BF16, name="ident", tag="ident")
            make_identity(nc, ident)
            wo_sb = [wpool.tile([128, D], BF16, name=f"wo{i}", tag=f"wo{i}")
                     for i in range(2)]

            qT = [big.tile([128, S], BF16, name=f"qT{i}", tag=f"qT{i}")
                  for i in range(2)]
            ckvT = [big.tile([128, S], BF16, name=f"ckvT{i}", tag=f"ckvT{i}")
                    for i in range(4)]
            kvT = [big.tile([128, S], BF16, name=f"kvT{i}", tag=f"kvT{i}")
                   for i in range(2)]
            kva = [big.tile([128, HC, DH + 1], BF16, name=f"kva{t}",
                            tag=f"kva{t}") for t in range(16)]
            attn_sb = [big.tile([128, HDC], BF16, name=f"attn{i}", tag=f"attn{i}")
                       for i in range(16)]
            attnT = [big.tile([128, S], BF16, name=f"attnT{i}", tag=f"attnT{i}")
                     for i in range(2)]

            def _u_ckv(rc, dlt):
                cs = slice(rc * 512, (rc + 1) * 512)
                ps = psA.tile([128, 512], F32, name="psB", tag="psB")
                for j in range(4):
                    nc.tensor.matmul(ps,
                                     wdkv_sb[:, 2 * j:2 * j + 2,
                                             dlt * 128:(dlt + 1) * 128],
                                     hTs[rc][:, 2 * j:2 * j + 2, :],
                                     start=(j == 0), stop=(j == 3),
                                     perf_mode=DR)
                nc.vector.tensor_scalar(out=ckvT[dlt][:, cs], in0=ps,
                                        scalar1=PRJ, scalar2=None,
                                        op0=mybir.AluOpType.mult)

            def _u_q(rc, ht):
                cs = slice(rc * 512, (rc + 1) * 512)
                ps = psA.tile([128, 512], F32, name="psB", tag="psB")
                for j in range(4):
                    nc.tensor.matmul(ps,
                                     wq_sb[:, 2 * j:2 * j + 2,
                                           ht * 128:(ht + 1) * 128],
                                     hTs[rc][:, 2 * j:2 * j + 2, :],
                                     start=(j == 0), stop=(j == 3),
                                     perf_mode=DR)
                nc.vector.tensor_scalar(out=qT[ht][:, cs], in0=ps,
                                        scalar1=PRJ, scalar2=None,
                                        op0=mybir.AluOpType.mult)

            def _u_kvT(rc, ht):
                cs = slice(rc * 512, (rc + 1) * 512)
                ps = psA.tile([128, 512], F32, name="psB", tag="psB")
                for dlt in range(4):
                    nc.tensor.matmul(ps,
                                     wukv_sb[dlt][:, ht * 128:(ht + 1) * 128],
                                     ckvT[dlt][:, cs],
                                     start=(dlt == 0), stop=(dlt == 3))
                nc.vector.tensor_copy(out=kvT[ht][:, cs], in_=ps)

            def _u_kva(rc, kt):
                nc.gpsimd.memset(kva[kt][:, :, DH:DH + 1], 1.0)
                ps = psA.tile([128, 512], F32, name="psB", tag="psB")
                for dlt in range(4):
                    nc.tensor.matmul(ps[:, 0:HDC],
                                     ckvT[dlt][:, kt * 128:(kt + 1) * 128],
                                     wukv_sb[dlt],
                                     start=(dlt == 0), stop=(dlt == 3))
                nc.vector.tensor_copy(
                    out=kva[kt][:, :, 0:DH],
                    in_=ps[:, 0:HDC].rearrange("p (h d) -> p h d", h=HC))

            def proj_units(rc):
                from functools import partial
                u = []
                for dlt in range(4):
                    u.append(partial(_u_ckv, rc, dlt))
                for ht in range(2):
                    u.append(partial(_u_q, rc, ht))
                for ht in range(2):
                    u.append(partial(_u_kvT, rc, ht))
                for kt in range(4 * rc, 4 * rc + 4):
                    u.append(partial(_u_kva, rc, kt))
                return u

            def attn(qp, inject):
                nkt = 2 * qp + 2
                Ps = {}

                def scores(h, g0):
                    gn = min(GSZ, nkt - g0)
                    tI, pO = h // 2, (h % 2) * 64
                    ps = psS.tile([128, 256 * GSZ], F32, name="psS", tag="psS")
                    for kl in range(gn):
                        kt = g0 + kl
                        nc.tensor.matmul(
                            ps[:, kl * 256:(kl + 1) * 256],
                            kvT[tI][pO:pO + 64, kt * 128:(kt + 1) * 128],
                            qT[tI][pO:pO + 64, qp * 256:(qp + 1) * 256],
                            start=True, stop=True)
                    pbT = work.tile([128, 256 * GSZ], BF16, name="pbT", tag="pbT")
                    nc.scalar.activation(out=pbT[:, 0:gn * 256],
                                         in_=ps[:, 0:gn * 256], func=AF.Exp,
                                         scale=1.0 / (DH ** 0.5))
                    for kl in range(gn):
                        kt = g0 + kl
                        if kt >= 2 * qp:  # diagonal: zero masked probs
                            nc.gpsimd.tensor_mul(
                                out=pbT[:, kl * 256:(kl + 1) * 256],
                                in0=pbT[:, kl * 256:(kl + 1) * 256],
                                in1=masks[:, kt - 2 * qp, :])
                    return pbT

                def pv(h, g0, pbT):
                    gn = min(GSZ, nkt - g0)
                    P0, P1 = Ps[h]
                    for kl in range(gn):
                        kt = g0 + kl
                        for j, P in ((0, P0), (1, P1)):
                            nc.tensor.matmul(
                                P,
                                pbT[:, kl * 256 + j * 128:kl * 256 + j * 128 + 128],
                                kva[kt][:, h, :],
                                start=(kt == 0), stop=(kt == nkt - 1))
                    if g0 + GSZ >= nkt:  # head finished: normalize + store
                        rec = work.tile([128, 2], F32, name="rec", tag="rec")
                        nc.vector.reciprocal(out=rec[:, 0:1], in_=P0[:, DH:DH + 1])
                        nc.vector.reciprocal(out=rec[:, 1:2], in_=P1[:, DH:DH + 1])
                        for j, P in ((0, P0), (1, P1)):
                            nc.vector.tensor_scalar_mul(
                                out=attn_sb[2 * qp + j][:, h * DH:(h + 1) * DH],
                                in0=P[:, 0:DH], scalar1=rec[:, j:j + 1])

                stages = [(h, g0) for h in range(HC) for g0 in range(0, nkt, GSZ)]
                pend = []
                for (h, g0) in stages:
                    if g0 == 0:
                        Ps[h] = (psO.tile([128, DH + 1], F32, name="P0",
                                          tag="Pacc"),
                                 psO.tile([128, DH + 1], F32, name="P1",
                                          tag="Pacc"))
                    pbT = scores(h, g0)
                    pend.append((h, g0, pbT))
                    if len(pend) > DPIPE:
                        pv(*pend.pop(0))
                    inject()  # slot proj work for the next rc into PE's queue
                for st in pend:
                    pv(*st)

            def post(qp):
                for qt in (2 * qp, 2 * qp + 1):
                    for hd in range(2):
                        pt = psA.tile([128, 128], BF16, name="ptT", tag="psB")
                        nc.tensor.transpose(
                            pt, attn_sb[qt][:, hd * 128:(hd + 1) * 128], ident)
                        nc.vector.tensor_copy(
                            out=attnT[hd][:, qt * 128:(qt + 1) * 128], in_=pt)
                    xp = work.tile([128, D], BF16, name="xp", tag="xp")
                    for dh2 in range(2):
                        ps = psA.tile([128, 512], F32, name="psB", tag="psB")
                        for hd in range(2):
                            nc.tensor.matmul(
                                ps, attnT[hd][:, qt * 128:(qt + 1) * 128],
                                wo_sb[hd][:, dh2 * 512:(dh2 + 1) * 512],
                                start=(hd == 0), stop=(hd == 1))
                        nc.vector.tensor_copy(
                            out=xp[:, dh2 * 512:(dh2 + 1) * 512], in_=ps)
                    nc.sync.dma_start(out=xpart[qt * 128:(qt + 1) * 128, :],
                                      in_=xp)

            for u in proj_units(0):
                u()
            for rc in range(1, 4):
                _load_h(rc)
            for i in range(2):
                nc.sync.dma_start(out=wo_sb[i], in_=wo[i * 128:(i + 1) * 128, :])
            for rc in range(4):
                nxt = iter(proj_units(rc + 1) if rc < 3 else [])

                def inject(k=1):
                    for _ in range(k):
                        u = next(nxt, None)
                        if u is not None:
                            u()

                for qp in (2 * rc, 2 * rc + 1):
                    # front-load injections so next-rc kv is ready on time
                    attn(qp, (lambda: inject(2)) if qp == 2 * rc else inject)
                    post(qp)
                for u in nxt:
                    u()
    nc.compile()
    return nc


F8 = mybir.dt.float8e4
DR = mybir.MatmulPerfMode.DoubleRow

# fp8 scale factors (powers of 2, folded back out via activation scale)
SX, SW1, SH, SW2 = 4.0, 64.0, 16.0, 32.0


def build_l2(capT: int):
    """Expert MLP on gathered tokens, fp8e4 DoubleRow matmuls.

    yT = gelu(Xe@W1 + b1) @ W2 / SW2  (un-combined); host applies the top-k
    combine weight and adds w*b2 during scatter. Xe is pre-scaled by SX,
    W1 by SW1, W2 by SW2 on the host.
    """
    nc = bacc.Bacc()
    xeT = nc.dram_tensor("xeT", [D, capT], F8, kind="ExternalInput")
    w1 = nc.dram_tensor("w1", [D, DFF], F8, kind="ExternalInput")
    w2 = nc.dram_tensor("w2", [DFF, D], F8, kind="ExternalInput")
    b1 = nc.dram_tensor("b1", [128, DFF // 128], F32, kind="ExternalInput")
    yT = nc.dram_tensor("yT", [D, capT], BF16, kind="ExternalOutput")

    chunks = [(0, min(256, capT))]
    off = chunks[0][1]
    while off < capT:
        n = min(512, capT - off)
        chunks.append((off, n))
        off += n

    with TileContext(nc) as tc:
        import contextlib
        with contextlib.ExitStack() as ctx:
            singles = ctx.enter_context(tc.tile_pool(name="singles", bufs=1))
            wpool = ctx.enter_context(tc.tile_pool(name="wpool", bufs=1))
            big = ctx.enter_context(tc.tile_pool(name="big", bufs=1))
            outp = ctx.enter_context(tc.tile_pool(name="outp", bufs=4))
            psp = ctx.enter_context(tc.tile_pool(name="psp", bufs=8, space="PSUM"))
            psq = psp

            # single-DMA loads: xe per chunk, w1 in two ft-halves, w2 whole,
            # issued in critical-path-first order
            xec = [big.tile([128, 8, n], F8, name=f"xe{c}", tag=f"xe{c}")
                   for c, (off, n) in enumerate(chunks)]
            w1s = [wpool.tile([128, 8, DFF // 4], F8, name=f"w1s{h}",
                              tag=f"w1s{h}") for h in range(4)]
            w2s = wpool.tile([128, 16, D], F8, name="w2s", tag="w2s")
            b1s = singles.tile([128, DFF // 128], F32, name="b1s", tag="b1s")

            def _load_xe(c):
                off, n = chunks[c]
                nc.sync.dma_start(
                    out=xec[c],
                    in_=xeT[:, off:off + n].rearrange(
                        "(j s p) n -> p (j s) n", j=4, s=2))

            _load_xe(0)
            q = DFF // 4
            nc.sync.dma_start(
                out=w1s[0],
                in_=w1[:, 0:q].rearrange("(j s p) n -> p (j s) n", j=4, s=2))
            nc.sync.dma_start(out=b1s, in_=b1[:, :])
            for h in range(1, 4):
                nc.sync.dma_start(
                    out=w1s[h],
                    in_=w1[:, h * q:(h + 1) * q].rearrange(
                        "(j s p) n -> p (j s) n", j=4, s=2))
            for c in range(1, len(chunks)):
                _load_xe(c)
            nc.sync.dma_start(
                out=w2s,
                in_=w2[:, :].rearrange("(j s p) n -> p (j s) n", j=8, s=2))

            hid2 = [[big.tile([128, 2, n], F8, name=f"hid2_{c}_{j}",
                              tag=f"hid2_{c}_{j}") for j in range(8)]
                    for c, (off, n) in enumerate(chunks)]
            for c, (off, n) in enumerate(chunks):
                for ft in range(16):
                    h, fl = ft // 4, ft % 4
                    ps = psp.tile([128, 512], F32, name="ps1", tag="ps")
                    for j in range(4):
                        nc.tensor.matmul(ps[:, 0:n],
                                         w1s[h][:, 2 * j:2 * j + 2,
                                                fl * 128:(fl + 1) * 128],
                                         xec[c][:, 2 * j:2 * j + 2, 0:n],
                                         start=(j == 0), stop=(j == 3),
                                         perf_mode=DR)
                    nc.scalar.activation(out=hid2[c][ft // 2][:, ft % 2, 0:n],
                                         in_=ps[:, 0:n], func=AF.Gelu,
                                         bias=b1s[:, ft:ft + 1],
                                         scale=1.0 / (SX * SW1))
            for c, (off, n) in enumerate(chunks):
                for dt in range(8):
                    ps = psq.tile([128, 512], F32, name="ps2", tag="ps")
                    for j in range(8):
                        nc.tensor.matmul(ps[:, 0:n],
                                         w2s[:, 2 * j:2 * j + 2,
                                             dt * 128:(dt + 1) * 128],
                                         hid2[c][j][:, :, 0:n],
                                         start=(j == 0), stop=(j == 7),
                                         perf_mode=DR)
                    ot = outp.tile([128, 512], BF16, name="ot", tag="ot")
                    if dt % 2 == 0:
                        nc.vector.tensor_scalar(out=ot[:, 0:n], in0=ps[:, 0:n],
                                                scalar1=1.0 / SW2, scalar2=None,
                                                op0=mybir.AluOpType.mult)
                    else:
                        nc.scalar.activation(out=ot[:, 0:n], in_=ps[:, 0:n],
                                             func=AF.Copy, scale=1.0 / SW2)
                    nc.sync.dma_start(out=yT[dt * 128:(dt + 1) * 128, off:off + n],
                                      in_=ot[:, 0:n])
    nc.compile()
    return nc


def _bf(a):
    return np.ascontiguousarray(np.asarray(a).astype(ml_dtypes.bfloat16))


def _f8(a, scale):
    a = np.asarray(a, np.float32) * scale
    np.clip(a, -240.0, 240.0, out=a)
    return np.ascontiguousarray(a.astype(ml_dtypes.float8_e4m3))


def _f32c(a):
    return np.ascontiguousarray(np.asarray(a, np.float32))


def kernel(x, mask, ln1_scale, ln1_bias, Wq, Wdkv, Wukv, Wo,
           ln2_scale, ln2_bias, Wgate, bgate, We1, be1, We2, be2,
           _collect=None):
    x = np.asarray(x, np.float32)
    g1v = np.asarray(ln1_scale, np.float32)
    b1v = np.asarray(ln1_bias, np.float32)

    # LN1 on host (elementwise prep); kernel gets hT pre-transposed in fp8
    mu = x.mean(axis=2, keepdims=True)
    var = ((x - mu) ** 2).mean(axis=2, keepdims=True)
    h1 = ((x - mu) / np.sqrt(var + EPS)) * g1v + b1v
    hT8 = [_f8(h1[b].T, SXA) for b in range(B)]

    # multiplicative 0/1 causal masks for the two diagonal 128k x 256q tiles
    ii = np.arange(128)[:, None]
    jj = np.arange(256)[None, :]
    m0 = (jj >= ii).astype(np.float32)
    m1 = (jj >= ii + 128).astype(np.float32)
    mmask = _bf(np.concatenate([m0, m1], axis=0))

    wdkv_8 = _f8(np.asarray(Wdkv, np.float32), SWP)
    l1_maps = []
    for c in range(8):
        b, g = c // 4, c % 4
        cs = slice(g * HDC, (g + 1) * HDC)
        l1_maps.append({
            "hT8": hT8[b],
            "wq": _f8(np.asarray(Wq, np.float32)[:, cs], SWP),
            "wdkv": wdkv_8,
            "wukv": _bf(np.asarray(Wukv)[:, cs]),
            "wo": _bf(np.asarray(Wo)[cs, :]),
            "mmask": mmask,
        })

    if "l1" not in _cache:
        _cache["l1"] = build_l1()
    r1 = run_bass_kernel_spmd(_cache["l1"], l1_maps, core_ids=list(range(8)))
    if _collect is not None:
        _collect["r1"] = r1

    xnew = x.copy().reshape(B, S, D)
    for c in range(8):
        xnew[c // 4] += r1.results[c]["xpart"].astype(np.float32)
    xf = xnew.reshape(B * S, D)

    # LN2 + gate on host (fp32)
    mu = xf.mean(axis=1, keepdims=True)
    var = ((xf - mu) ** 2).mean(axis=1, keepdims=True)
    h2 = ((xf - mu) / np.sqrt(var + EPS) * np.asarray(ln2_scale, np.float32)
          + np.asarray(ln2_bias, np.float32)).astype(np.float32)
    logits = h2 @ np.asarray(Wgate, np.float32) + np.asarray(bgate, np.float32)
    order = np.argsort(-logits, axis=1, kind="stable")[:, :TOPK]
    tv = np.take_along_axis(logits, order, axis=1)
    ex = np.exp(tv - tv.max(axis=1, keepdims=True))
    wtop = (ex / ex.sum(axis=1, keepdims=True)).astype(np.float32)

    idxs, wts = [], []
    for e in range(E):
        m_e = (order == e)
        rows = np.nonzero(m_e.any(axis=1))[0]
        w_e = (wtop * m_e).sum(axis=1)[rows]
        idxs.append(rows)
        wts.append(w_e.astype(np.float32))
    maxc = max(len(r) for r in idxs)
    capT = max(512, ((maxc + 127) // 128) * 128)

    w1_b, w2_b = np.asarray(We1), np.asarray(We2)
    be1_f, be2_f = np.asarray(be1, np.float32), np.asarray(be2, np.float32)
    l2_maps = []
    for e in range(E):
        n = len(idxs[e])
        xeT = np.zeros((D, capT), ml_dtypes.float8_e4m3)
        xeT[:, :n] = _f8(h2[idxs[e]].T, SX)
        l2_maps.append({
            "xeT": np.ascontiguousarray(xeT),
            "w1": _f8(w1_b[e], SW1),
            "w2": _f8(w2_b[e], SW2),
            "b1": np.ascontiguousarray(be1_f[e].reshape(DFF // 128, 128).T),
        })

    key = ("l2", capT)
    if key not in _cache:
        _cache[key] = build_l2(capT)
    r2 = run_bass_kernel_spmd(_cache[key], l2_maps, core_ids=list(range(8)))
    if _collect is not None:
        _collect["r2"] = r2

    out = xf.copy()
    for e in range(E):
        n = len(idxs[e])
        out[idxs[e]] += wts[e][:, None] * (
            r2.results[e]["yT"][:, :n].T.astype(np.float32)
            + be2_f[e][None, :])
    return out.reshape(B, S, D).astype(np.float32)



# revision 53
# speedup vs baseline: 2.4860x; 1.0452x over previous
"""Trainium2 Bass kernel for a transformer block: MLA attention + top-2 MoE (8 experts).

Sharding (8 NeuronCores):
  Launch 1 (head-parallel attention): core c = (batch b=c//4, head-group
    g=c%4 of 4 heads). LN1 scale/bias are folded into host-scaled weights +
    per-partition projection biases. Causal attention uses a transposed-scores
    layout with softmax denominators accumulated via an augmented ones column.
    Partial out-projection attn_g @ Wo[g-rows] per core; host sums partials.
  Host: xnew = x + sum(partials); LN2; gate logits; top-2 softmax; per-expert
    token gather (the "all-to-all dispatch").
  Launch 2 (expert-parallel MLP): core e = expert e on its gathered tokens,
    combine weights folded in. Host scatter-adds ("combine").
"""

import numpy as np
import ml_dtypes

import concourse.bass as bass
import concourse.bacc as bacc
import concourse.mybir as mybir
from concourse.tile import TileContext
from concourse.masks import make_identity
from concourse.bass_utils import run_bass_kernel_spmd

F32 = mybir.dt.float32
BF16 = mybir.dt.bfloat16
AF = mybir.ActivationFunctionType

B, S, D = 2, 2048, 1024
H, DH, DL = 16, 64, 512
E, DFF, TOPK = 8, 2048, 2
HC = 4            # heads per core
HDC = HC * DH     # 256
EPS = 1e-5
NEG = -1.0e30

_cache = {}


# l1 fp8 scales: h ×SXA, Wq/Wdkv ×SWP (q/ckv psum = SXA*SWP * true)
SXA, SWP = 4.0, 64.0
SAT, SWO = 4.0, 64.0   # attn out x4 into fp8; Wo x64 into fp8
GSZ = 4     # k-tiles per softmax group (psS bank budget)
DPIPE = 2   # scores-ahead-of-PV software pipeline depth


def build_l1():
    """Head-parallel MLA attention. Host pre-computes LN1 and passes hT fp8.

    Per core: q/ckv projections in fp8 DoubleRow; kv up-projection in bf16
    (both layouts computed on PE, no DRAM roundtrip); causal softmax with
    multiplicative 0/1 masks applied to exp(scores); denominators via an
    augmented ones column; out-projection partials DMAed straight from PSUM.
    """
    nc = bacc.Bacc()
    hT8 = nc.dram_tensor("hT8", [D, S], F8, kind="ExternalInput")
    wq = nc.dram_tensor("wq", [D, HDC], F8, kind="ExternalInput")
    wdkv = nc.dram_tensor("wdkv", [D, DL], F8, kind="ExternalInput")
    wukv = nc.dram_tensor("wukv", [DL, HDC], BF16, kind="ExternalInput")
    wo = nc.dram_tensor("wo", [HDC, D], F8, kind="ExternalInput")
    mmask = nc.dram_tensor("mmask", [2 * 128, 256], F8, kind="ExternalInput")
    xpart = nc.dram_tensor("xpart", [S, D], BF16, kind="ExternalOutput")

    PRJ = 1.0 / (SXA * SWP)

    with TileContext(nc) as tc:
        import contextlib
        with contextlib.ExitStack() as ctx:
            singles = ctx.enter_context(tc.tile_pool(name="singles", bufs=1))
            wpool = ctx.enter_context(tc.tile_pool(name="wpool", bufs=1))
            big = ctx.enter_context(tc.tile_pool(name="big", bufs=1))
            work = ctx.enter_context(tc.tile_pool(name="work", bufs=6))
            psA = ctx.enter_context(tc.tile_pool(name="psA", bufs=2, space="PSUM"))
            psS = ctx.enter_context(tc.tile_pool(name="psS", bufs=2, space="PSUM"))
            psO = ctx.enter_context(tc.tile_pool(name="psO", bufs=2, space="PSUM"))

            # loads, critical-path first
            hTs = [big.tile([128, 8, 512], F8, name=f"hT{rc}", tag=f"hT{rc}")
                   for rc in range(4)]

            def _load_h(rc):
                nc.sync.dma_start(
                    out=hTs[rc],
                    in_=hT8[:, rc * 512:(rc + 1) * 512].rearrange(
                        "(j s p) n -> p (j s) n", j=4, s=2))

            _load_h(0)
            wdkv_sb = wpool.tile([128, 8, DL], F8, name="wdkv", tag="wdkv")
            nc.sync.dma_start(
                out=wdkv_sb, in_=wdkv[:, :].rearrange("(j s p) n -> p (j s) n",
                                                      j=4, s=2))
            wukv_sb = [wpool.tile([128, HDC], BF16, name=f"wukv{i}",
                                  tag=f"wukv{i}") for i in range(4)]
            for i in range(4):
                nc.sync.dma_start(out=wukv_sb[i],
                                  in_=wukv[i * 128:(i + 1) * 128, :])
            wq_sb = wpool.tile([128, 8, HDC], F8, name="wq", tag="wq")
            nc.sync.dma_start(
                out=wq_sb, in_=wq[:, :].rearrange("(j s p) n -> p (j s) n",
                                                  j=4, s=2))
            masks = singles.tile([128, 2, 256], F8, name="masks", tag="masks")
            nc.sync.dma_start(out=masks,
                              in_=mmask[:, :].rearrange("(v p) n -> p v n", v=2))
            ident = singles.tile([128, 128], BF16, name="ident", tag="ident")
            make_identity(nc, ident)
            wo_sb = wpool.tile([128, 2, D], F8, name="wo", tag="wo")

            qT = [big.tile([128, S], BF16, name=f"qT{i}", tag=f"qT{i}")
                  for i in range(2)]
            ckvT = [big.tile([128, S], BF16, name=f"ckvT{i}", tag=f"ckvT{i}")
                    for i in range(4)]
            kvT = [big.tile([128, S], BF16, name=f"kvT{i}", tag=f"kvT{i}")
                   for i in range(2)]
            kva2 = [big.tile([128, 2, HC, DH + 1], F8, name=f"kva{t}",
                             tag=f"kva{t}") for t in range(8)]
            attn_sb = [big.tile([128, HDC], BF16, name=f"attn{i}", tag=f"attn{i}")
                       for i in range(16)]
            attnT2 = big.tile([128, 2, S], F8, name="attnT2", tag="attnT2")

            def _u_ckv(rc, dlt):
                cs = slice(rc * 512, (rc + 1) * 512)
                ps = psA.tile([128, 512], F32, name="psB", tag="psB")
                for j in range(4):
                    nc.tensor.matmul(ps,
                                     wdkv_sb[:, 2 * j:2 * j + 2,
                                             dlt * 128:(dlt + 1) * 128],
                                     hTs[rc][:, 2 * j:2 * j + 2, :],
                                     start=(j == 0), stop=(j == 3),
                                     perf_mode=DR)
                nc.vector.tensor_scalar(out=ckvT[dlt][:, cs], in0=ps,
                                        scalar1=PRJ, scalar2=None,
                                        op0=mybir.AluOpType.mult)

            def _u_q(rc, ht):
                cs = slice(rc * 512, (rc + 1) * 512)
                ps = psA.tile([128, 512], F32, name="psB", tag="psB")
                for j in range(4):
                    nc.tensor.matmul(ps,
                                     wq_sb[:, 2 * j:2 * j + 2,
                                           ht * 128:(ht + 1) * 128],
                                     hTs[rc][:, 2 * j:2 * j + 2, :],
                                     start=(j == 0), stop=(j == 3),
                                     perf_mode=DR)
                nc.vector.tensor_scalar(out=qT[ht][:, cs], in0=ps,
                                        scalar1=PRJ, scalar2=None,
                                        op0=mybir.AluOpType.mult)

            def _u_kvT(rc, ht):
                cs = slice(rc * 512, (rc + 1) * 512)
                ps = psA.tile([128, 512], F32, name="psB", tag="psB")
                for dlt in range(4):
                    nc.tensor.matmul(ps,
                                     wukv_sb[dlt][:, ht * 128:(ht + 1) * 128],
                                     ckvT[dlt][:, cs],
                                     start=(dlt == 0), stop=(dlt == 3))
                nc.vector.tensor_copy(out=kvT[ht][:, cs], in_=ps)

            def _u_kva(rc, kt):
                kp, ks = kt // 2, kt % 2
                nc.gpsimd.memset(kva2[kp][:, ks, :, DH:DH + 1], 1.0)
                ps = psA.tile([128, 512], F32, name="psB", tag="psB")
                for dlt in range(4):
                    nc.tensor.matmul(ps[:, 0:HDC],
                                     ckvT[dlt][:, kt * 128:(kt + 1) * 128],
                                     wukv_sb[dlt],
                                     start=(dlt == 0), stop=(dlt == 3))
                nc.vector.tensor_copy(
                    out=kva2[kp][:, ks, :, 0:DH],
                    in_=ps[:, 0:HDC].rearrange("p (h d) -> p h d", h=HC))

            def proj_units(rc):
                from functools import partial
                u = []
                for dlt in range(4):
                    u.append(partial(_u_ckv, rc, dlt))
                for ht in range(2):
                    u.append(partial(_u_q, rc, ht))
                for ht in range(2):
                    u.append(partial(_u_kvT, rc, ht))
                for kt in range(4 * rc, 4 * rc + 4):
                    u.append(partial(_u_kva, rc, kt))
                return u

            def attn(qp, inject):
                nkt = 2 * qp + 2
                Ps = {}

                def scores(h, g0):
                    gn = min(GSZ, nkt - g0)
                    tI, pO = h // 2, (h % 2) * 64
                    ps = psS.tile([128, 256 * GSZ], F32, name="psS", tag="psS")
                    for kl in range(gn):
                        kt = g0 + kl
                        nc.tensor.matmul(
                            ps[:, kl * 256:(kl + 1) * 256],
                            kvT[tI][pO:pO + 64, kt * 128:(kt + 1) * 128],
                            qT[tI][pO:pO + 64, qp * 256:(qp + 1) * 256],
                            start=True, stop=True)
                    pbT = work.tile([128, 256 * GSZ], F8, name="pbT", tag="pbT")
                    nc.scalar.activation(out=pbT[:, 0:gn * 256],
                                         in_=ps[:, 0:gn * 256], func=AF.Exp,
                                         scale=1.0 / (DH ** 0.5))
                    for kl in range(gn):
                        kt = g0 + kl
                        if kt >= 2 * qp:  # diagonal: zero masked probs
                            nc.gpsimd.tensor_mul(
                                out=pbT[:, kl * 256:(kl + 1) * 256],
                                in0=pbT[:, kl * 256:(kl + 1) * 256],
                                in1=masks[:, kt - 2 * qp, :])
                    return pbT

                def pv(h, g0, pbT):
                    gn = min(GSZ, nkt - g0)
                    P0, P1 = Ps[h]
                    vk = pbT.rearrange("p (k j c) -> p k j c", k=GSZ, j=2)
                    for kl in range(0, gn, 2):
                        kt = g0 + kl
                        for j, P in ((0, P0), (1, P1)):
                            nc.tensor.matmul(
                                P,
                                vk[:, kl:kl + 2, j, :],
                                kva2[kt // 2][:, :, h, :],
                                start=(kt == 0), stop=(kt + 2 == nkt),
                                perf_mode=DR)
                    if g0 + GSZ >= nkt:  # head finished: normalize + store
                        rec = work.tile([128, 2], F32, name="rec", tag="rec")
                        nc.vector.reciprocal(out=rec[:, 0:1], in_=P0[:, DH:DH + 1])
                        nc.vector.reciprocal(out=rec[:, 1:2], in_=P1[:, DH:DH + 1])
                        for j, P in ((0, P0), (1, P1)):
                            nc.vector.tensor_scalar_mul(
                                out=attn_sb[2 * qp + j][:, h * DH:(h + 1) * DH],
                                in0=P[:, 0:DH], scalar1=rec[:, j:j + 1])

                stages = [(h, g0) for h in range(HC) for g0 in range(0, nkt, GSZ)]
                pend = []
                for (h, g0) in stages:
                    if g0 == 0:
                        Ps[h] = (psO.tile([128, DH + 1], F32, name="P0",
                                          tag="Pacc"),
                                 psO.tile([128, DH + 1], F32, name="P1",
                                          tag="Pacc"))
                    pbT = scores(h, g0)
                    pend.append((h, g0, pbT))
                    if len(pend) > DPIPE:
                        pv(*pend.pop(0))
                    inject()  # slot proj work for the next rc into PE's queue
                for st in pend:
                    pv(*st)

            def post(qp):
                for qt in (2 * qp, 2 * qp + 1):
                    for hd in range(2):
                        pt = psA.tile([128, 128], BF16, name="ptT", tag="psB")
                        nc.tensor.transpose(
                            pt, attn_sb[qt][:, hd * 128:(hd + 1) * 128], ident)
                        nc.vector.tensor_scalar(
                            out=attnT2[:, hd, qt * 128:(qt + 1) * 128], in0=pt,
                            scalar1=SAT, scalar2=None,
                            op0=mybir.AluOpType.mult)
                    xp = work.tile([128, D], BF16, name="xp", tag="xp")
                    for dh2 in range(2):
                        ps = psA.tile([128, 512], F32, name="psB", tag="psB")
                        nc.tensor.matmul(
                            ps, attnT2[:, :, qt * 128:(qt + 1) * 128],
                            wo_sb[:, :, dh2 * 512:(dh2 + 1) * 512],
                            start=True, stop=True, perf_mode=DR)
                        nc.vector.tensor_scalar(
                            out=xp[:, dh2 * 512:(dh2 + 1) * 512], in0=ps,
                            scalar1=1.0 / (SAT * SWO), scalar2=None,
                            op0=mybir.AluOpType.mult)
                    nc.sync.dma_start(out=xpart[qt * 128:(qt + 1) * 128, :],
                                      in_=xp)

            for u in proj_units(0):
                u()
            for rc in range(1, 4):
                _load_h(rc)
            nc.sync.dma_start(
                out=wo_sb,
                in_=wo[:, :].rearrange("(s p) n -> p s n", s=2))
            for rc in range(4):
                nxt = iter(proj_units(rc + 1) if rc < 3 else [])

                def inject(k=1):
                    for _ in range(k):
                        u = next(nxt, None)
                        if u is not None:
                            u()

                for qp in (2 * rc, 2 * rc + 1):
                    # front-load injections so next-rc kv is ready on time
                    attn(qp, (lambda: inject(2)) if qp == 2 * rc else inject)
                    post(qp)
                for u in nxt:
                    u()
    nc.compile()
    return nc


F8 = mybir.dt.float8e4
DR = mybir.MatmulPerfMode.DoubleRow

# fp8 scale factors (powers of 2, folded back out via activation scale)
SX, SW1, SH, SW2 = 4.0, 64.0, 16.0, 32.0


def build_l2(capT: int):
    """Expert MLP on gathered tokens, fp8e4 DoubleRow matmuls.

    yT = gelu(Xe@W1 + b1) @ W2 / SW2  (un-combined); host applies the top-k
    combine weight and adds w*b2 during scatter. Xe is pre-scaled by SX,
    W1 by SW1, W2 by SW2 on the host.
    """
    nc = bacc.Bacc()
    xeT = nc.dram_tensor("xeT", [D, capT], F8, kind="ExternalInput")
    w1 = nc.dram_tensor("w1", [D, DFF], F8, kind="ExternalInput")
    w2 = nc.dram_tensor("w2", [DFF, D], F8, kind="ExternalInput")
    b1 = nc.dram_tensor("b1", [128, DFF // 128], F32, kind="ExternalInput")
    yT = nc.dram_tensor("yT", [D, capT], BF16, kind="ExternalOutput")

    chunks = [(0, min(256, capT))]
    off = chunks[0][1]
    while off < capT:
        n = min(512, capT - off)
        chunks.append((off, n))
        off += n

    with TileContext(nc) as tc:
        import contextlib
        with contextlib.ExitStack() as ctx:
            singles = ctx.enter_context(tc.tile_pool(name="singles", bufs=1))
            wpool = ctx.enter_context(tc.tile_pool(name="wpool", bufs=1))
            big = ctx.enter_context(tc.tile_pool(name="big", bufs=1))
            outp = ctx.enter_context(tc.tile_pool(name="outp", bufs=4))
            psp = ctx.enter_context(tc.tile_pool(name="psp", bufs=8, space="PSUM"))
            psq = psp

            # single-DMA loads: xe per chunk, w1 in two ft-halves, w2 whole,
            # issued in critical-path-first order
            xec = [big.tile([128, 8, n], F8, name=f"xe{c}", tag=f"xe{c}")
                   for c, (off, n) in enumerate(chunks)]
            w1s = [wpool.tile([128, 8, DFF // 4], F8, name=f"w1s{h}",
                              tag=f"w1s{h}") for h in range(4)]
            w2s = wpool.tile([128, 16, D], F8, name="w2s", tag="w2s")
            b1s = singles.tile([128, DFF // 128], F32, name="b1s", tag="b1s")

            def _load_xe(c):
                off, n = chunks[c]
                nc.sync.dma_start(
                    out=xec[c],
                    in_=xeT[:, off:off + n].rearrange(
                        "(j s p) n -> p (j s) n", j=4, s=2))

            _load_xe(0)
            q = DFF // 4
            nc.sync.dma_start(
                out=w1s[0],
                in_=w1[:, 0:q].rearrange("(j s p) n -> p (j s) n", j=4, s=2))
            nc.sync.dma_start(out=b1s, in_=b1[:, :])
            for h in range(1, 4):
                nc.sync.dma_start(
                    out=w1s[h],
                    in_=w1[:, h * q:(h + 1) * q].rearrange(
                        "(j s p) n -> p (j s) n", j=4, s=2))
            for c in range(1, len(chunks)):
                _load_xe(c)
            nc.sync.dma_start(
                out=w2s,
                in_=w2[:, :].rearrange("(j s p) n -> p (j s) n", j=8, s=2))

            hid2 = [[big.tile([128, 2, n], F8, name=f"hid2_{c}_{j}",
                              tag=f"hid2_{c}_{j}") for j in range(8)]
                    for c, (off, n) in enumerate(chunks)]
            for c, (off, n) in enumerate(chunks):
                for ft in range(16):
                    h, fl = ft // 4, ft % 4
                    ps = psp.tile([128, 512], F32, name="ps1", tag="ps")
                    for j in range(4):
                        nc.tensor.matmul(ps[:, 0:n],
                                         w1s[h][:, 2 * j:2 * j + 2,
                                                fl * 128:(fl + 1) * 128],
                                         xec[c][:, 2 * j:2 * j + 2, 0:n],
                                         start=(j == 0), stop=(j == 3),
                                         perf_mode=DR)
                    nc.scalar.activation(out=hid2[c][ft // 2][:, ft % 2, 0:n],
                                         in_=ps[:, 0:n], func=AF.Gelu,
                                         bias=b1s[:, ft:ft + 1],
                                         scale=1.0 / (SX * SW1))
            for c, (off, n) in enumerate(chunks):
                for dt in range(8):
                    ps = psq.tile([128, 512], F32, name="ps2", tag="ps")
                    for j in range(8):
                        nc.tensor.matmul(ps[:, 0:n],
                                         w2s[:, 2 * j:2 * j + 2,
                                             dt * 128:(dt + 1) * 128],
                                         hid2[c][j][:, :, 0:n],
                                         start=(j == 0), stop=(j == 7),
                                         perf_mode=DR)
                    ot = outp.tile([128, 512], BF16, name="ot", tag="ot")
                    if dt % 2 == 0:
                        nc.vector.tensor_scalar(out=ot[:, 0:n], in0=ps[:, 0:n],
                                                scalar1=1.0 / SW2, scalar2=None,
                                                op0=mybir.AluOpType.mult)
                    else:
                        nc.scalar.activation(out=ot[:, 0:n], in_=ps[:, 0:n],
                                             func=AF.Copy, scale=1.0 / SW2)
                    nc.sync.dma_start(out=yT[dt * 128:(dt + 1) * 128, off:off + n],
                                      in_=ot[:, 0:n])
    nc.compile()
    return nc


def _bf(a):
    return np.ascontiguousarray(np.asarray(a).astype(ml_dtypes.bfloat16))


def _f8(a, scale):
    a = np.asarray(a, np.float32) * scale
    np.clip(a, -240.0, 240.0, out=a)
    return np.ascontiguousarray(a.astype(ml_dtypes.float8_e4m3))


def _f32c(a):
    return np.ascontiguousarray(np.asarray(a, np.float32))


def kernel(x, mask, ln1_scale, ln1_bias, Wq, Wdkv, Wukv, Wo,
           ln2_scale, ln2_bias, Wgate, bgate, We1, be1, We2, be2,
           _collect=None):
    x = np.asarray(x, np.float32)
    g1v = np.asarray(ln1_scale, np.float32)
    b1v = np.asarray(ln1_bias, np.float32)

    # LN1 on host (elementwise prep); kernel gets hT pre-transposed in fp8
    mu = x.mean(axis=2, keepdims=True)
    var = ((x - mu) ** 2).mean(axis=2, keepdims=True)
    h1 = ((x - mu) / np.sqrt(var + EPS)) * g1v + b1v
    hT8 = [_f8(h1[b].T, SXA) for b in range(B)]

    # multiplicative 0/1 causal masks for the two diagonal 128k x 256q tiles
    ii = np.arange(128)[:, None]
    jj = np.arange(256)[None, :]
    m0 = (jj >= ii).astype(np.float32)
    m1 = (jj >= ii + 128).astype(np.float32)
    mmask = np.ascontiguousarray(
        np.concatenate([m0, m1], axis=0).astype(ml_dtypes.float8_e4m3))

    wdkv_8 = _f8(np.asarray(Wdkv, np.float32), SWP)
    l1_maps = []
    for c in range(8):
        b, g = c // 4, c % 4
        cs = slice(g * HDC, (g + 1) * HDC)
        l1_maps.append({
            "hT8": hT8[b],
            "wq": _f8(np.asarray(Wq, np.float32)[:, cs], SWP),
            "wdkv": wdkv_8,
            "wukv": _bf(np.asarray(Wukv)[:, cs]),
            "wo": _f8(np.asarray(Wo, np.float32)[cs, :], SWO),
            "mmask": mmask,
        })

    if "l1" not in _cache:
        _cache["l1"] = build_l1()
    r1 = run_bass_kernel_spmd(_cache["l1"], l1_maps, core_ids=list(range(8)))
    if _collect is not None:
        _collect["r1"] = r1

    xnew = x.copy().reshape(B, S, D)
    for c in range(8):
        xnew[c // 4] += r1.results[c]["xpart"].astype(np.float32)
    xf = xnew.reshape(B * S, D)

    # LN2 + gate on host (fp32)
    mu = xf.mean(axis=1, keepdims=True)
    var = ((xf - mu) ** 2).mean(axis=1, keepdims=True)
    h2 = ((xf - mu) / np.sqrt(var + EPS) * np.asarray(ln2_scale, np.float32)
          + np.asarray(ln2_bias, np.float32)).astype(np.float32)
    logits = h2 @ np.asarray(Wgate, np.float32) + np.asarray(bgate, np.float32)
    order = np.argsort(-logits, axis=1, kind="stable")[:, :TOPK]
    tv = np.take_along_axis(logits, order, axis=1)
    ex = np.exp(tv - tv.max(axis=1, keepdims=True))
    wtop = (ex / ex.sum(axis=1, keepdims=True)).astype(np.float32)

    idxs, wts = [], []
    for e in range(E):
        m_e = (order == e)
        rows = np.nonzero(m_e.any(axis=1))[0]
        w_e = (wtop * m_e).sum(axis=1)[rows]
        idxs.append(rows)
        wts.append(w_e.astype(np.float32))
    maxc = max(len(r) for r in idxs)
    capT = max(512, ((maxc + 127) // 128) * 128)

    w1_b, w2_b = np.asarray(We1), np.asarray(We2)
    be1_f, be2_f = np.asarray(be1, np.float32), np.asarray(be2, np.float32)
    l2_maps = []
    for e in range(E):
        n = len(idxs[e])
        xeT = np.zeros((D, capT), ml_dtypes.float8_e4m3)
        xeT[:, :n] = _f8(h2[idxs[e]].T, SX)
        l2_maps.append({
            "xeT": np.ascontiguousarray(xeT),
            "w1": _f8(w1_b[e], SW1),
            "w2": _f8(w2_b[e], SW2),
            "b1": np.ascontiguousarray(be1_f[e].reshape(DFF // 128, 128).T),
        })

    key = ("l2", capT)
    if key not in _cache:
        _cache[key] = build_l2(capT)
    r2 = run_bass_kernel_spmd(_cache[key], l2_maps, core_ids=list(range(8)))
    if _collect is not None:
        _collect["r2"] = r2

    out = xf.copy()
    for e in range(E):
        n = len(idxs[e])
        out[idxs[e]] += wts[e][:, None] * (
            r2.results[e]["yT"][:, :n].T.astype(np.float32)
            + be2_f[e][None, :])
    return out.reshape(B, S, D).astype(np.float32)

